# revision 1
# baseline (speedup 1.0000x reference)
"""MoE (top-2 of 8 experts) forward on 8 Trainium2 NeuronCores.

Strategy (expert parallel, per the sharding hint):
  - core c owns expert c (w1[c], w2[c] are the only sharded inputs,
    passed bf16; everything else is replicated).
  - every core computes the full routing on device (logits -> top2 ->
    normalized weights -> per-expert counting sort via strict-triangular
    prefix matmuls), so the dispatch "all-to-all" is a local indirect-DMA
    gather of the ~512 token copies routed to the core's expert.
  - the compact MLP (two matmuls + silu, bf16 operands / fp32 psum
    accumulate) runs on the compacted tokens with all weights resident
    in SBUF; rows are scaled by their routing weight.
  - the weighted rows are scattered into an AllToAll send buffer laid
    out [dest_slab, CAP]; the return-path AllToAll (raw-bass tail, this
    container's walrus cannot compile collectives inside TileContext)
    delivers each row to the core owning that token's 256-row output
    slab, which gathers its two rows per token and adds them.
  - core c outputs rows [256c, 256c+256); the host only concatenates.

kernel(**inputs) -> full [2048, 768] float32 output.
"""
import os
import sys

sys.path.insert(0, "/opt/trn_rl_repo")

import numpy as np

import concourse.bass as bass
import concourse.mybir as mybir
import concourse.tile as tile
from concourse.bass import IndirectOffsetOnAxis

F32 = mybir.dt.float32
BF16 = mybir.dt.bfloat16
I32 = mybir.dt.int32
U32 = mybir.dt.uint32
AF = mybir.ActivationFunctionType
OP = mybir.AluOpType
AX = mybir.AxisListType

T, H, E, K, F = 2048, 768, 8, 2, 3072
P = 128
NCORE = 8
NT = T // P          # 16 token tiles
NH = H // P          # 6 hidden chunks
NF = F // P          # 24 ffn chunks
C = 768              # compact-list capacity per expert (mean 512, +12 sigma)
NC = C // P          # 6 compact tiles
CAP = 96             # capacity per (expert, slab) cell (mean 64, obs max 82)
SEND_ROWS = NCORE * CAP          # 768 rows in the a2a payload
SEND_FULL = 1024                 # send buffer incl. trash rows
BIG = 8192.0
SLAB = T // NCORE    # 256 tokens per output slab

COMBINE = os.environ.get("MOE_COMBINE", "a2a_raw")  # a2a_raw | hostsum

# ---------------------------------------------------------------------------
# This container's walrus cannot attach sem-wait commands to most
# instruction types. Two workarounds (see _split_attached_waits and the
# patched kernel-tail below): waits are moved onto standalone
# EventSemaphore instructions, and the Tile tail drain's ~20 waits are
# split across a chain of SP nops.
_MAX_WAITS = 4


def _patched_drain_and_barrier(self, tick_clock, wait_clock):
    from concourse.tile import ScopedClock, VectorClock
    from concourse.tile_sem_assignment import N_PROCS

    g = tick_clock.global_clock
    ticks = [g[p] for p in range(N_PROCS)]
    procs = [p for p in range(N_PROCS) if ticks[p] > 0]
    observed = [0] * N_PROCS
    for i in range(0, len(procs), _MAX_WAITS):
        chunk = set(procs[i : i + _MAX_WAITS])
        part = VectorClock([ticks[p] if p in chunk else 0 for p in range(N_PROCS)])
        nop = self.nc.sync.nop()
        wait_clock.add_sem_waits(
            nop.ins,
            ScopedClock({None: part}),
            ScopedClock({None: VectorClock(list(observed))}),
        )
        for p in chunk:
            observed[p] = ticks[p]
    drain_inst = self.nc.sync.drain()
    wait_clock.add_sem_waits(
        drain_inst.ins,
        ScopedClock({None: g}),
        ScopedClock({None: VectorClock(list(observed))}),
    )
    self.nc.all_engine_barrier()
    assert self.sems is not None
    popped = self.nc._tile_sem_poison_stack.pop()
    assert popped is self._sem_poison
    self.nc.clear_and_free_semaphores(list(self.sems.allocated().values()))
    self.nc.all_engine_barrier()


tile.TileContext._drain_and_barrier = _patched_drain_and_barrier


def _split_attached_waits(nc):
    n = 0
    for f in nc.m.functions:
        for bb in f.blocks:
            new = []
            for inst in bb.instructions:
                si = getattr(inst, "sync_info", None)
                waits = list(si.on_wait) if (si and si.on_wait) else []
                if waits and not isinstance(inst, mybir.InstEventSemaphore):
                    for k, w in enumerate(waits):
                        n += 1
                        new.append(
                            mybir.InstEventSemaphore(
                                name=f"{inst.name}-w{k}",
                                engine=inst.engine,
                                ins=[],
                                outs=[],
                                sync_info=mybir.SyncInfo(on_wait=[w], on_update=[]),
                            )
                        )
                    si.on_wait = []
                new.append(inst)
            bb.instructions[:] = new
    return n


def build_nc(combine=COMBINE):
    nc = bass.Bass(num_devices=NCORE)
    x_d = nc.declare_dram_parameter("x", [T, H], F32, isOutput=False)
    rw_d = nc.declare_dram_parameter("rw", [H, E], F32, isOutput=False)
    w1_d = nc.declare_dram_parameter("w1c", [H, F], BF16, isOutput=False)
    w2_d = nc.declare_dram_parameter("w2c", [F, H], BF16, isOutput=False)
    # host-precomputed constants (avoid gpsimd iota/affine at kernel start)
    id_d = nc.declare_dram_parameter("identc", [P, P], F32, isOutput=False)
    u_d = nc.declare_dram_parameter("ustrict", [P, P], F32, isOutput=False)
    ec_d = nc.declare_dram_parameter("ecolA", [P, NT * E], F32, isOutput=False)
    oh_d = nc.declare_dram_parameter("onehotA", [P, NT * E], F32, isOutput=False)
    tk_d = nc.declare_dram_parameter("tokfA", [P, NT], F32, isOutput=False)
    vs_d = nc.declare_dram_parameter("vslabA", [P, NT], F32, isOutput=False)
    sb_d = nc.declare_dram_parameter("slabbase", [P, 1], F32, isOutput=False)
    if combine == "hostsum":
        out_d = nc.declare_dram_parameter("out", [T, H], F32, isOutput=True)
    else:
        out_d = nc.declare_dram_parameter("out", [SLAB, H], F32, isOutput=True)

    # plain DRAM scratch shared with the raw tail
    send_dram = nc.dram_tensor("send_buf", [SEND_FULL, H], BF16)
    offs_dram = nc.dram_tensor("offs_buf", [T, 2], F32)
    recv_dram = nc.dram_tensor("recv_buf", [SEND_ROWS, H], BF16)

    tc = tile.TileContext(nc)
    with tc:
        with (
            tc.tile_pool(name="dram", bufs=1, space="DRAM") as dr,
            tc.tile_pool(name="consts", bufs=1) as cb,
            tc.tile_pool(name="weights", bufs=1) as wp,
            tc.tile_pool(name="work", bufs=2) as wk,
            tc.tile_pool(name="psum", bufs=2, space="PSUM") as ps,
        ):
            listbufs = [
                dr.tile([C, 3], F32, tag=f"listbuf{i}", name=f"listbuf{i}")
                for i in range(NT)
            ]
            if combine == "hostsum":
                part_dram = dr.tile([2176, H], F32, tag="part")

            # ---- constants (DMA'd or DVE-built; nothing on gpsimd) ----
            ident = cb.tile([P, P], F32, tag="ident")
            nc.sync.dma_start(ident, id_d[:, :])
            ident_bf = cb.tile([P, P], BF16, tag="ident_bf")
            nc.vector.tensor_copy(ident_bf, ident)
            U = cb.tile([P, P], F32, tag="ustrict")
            nc.sync.dma_start(U, u_d[:, :])
            ecolA = cb.tile([P, NT, E], F32, tag="ecolA")
            nc.sync.dma_start(ecolA, ec_d[:, :].rearrange("p (i e) -> p i e", e=E))
            onehotA = cb.tile([P, NT, E], F32, tag="onehotA")
            nc.sync.dma_start(onehotA, oh_d[:, :].rearrange("p (i e) -> p i e", e=E))
            tokfA = cb.tile([P, NT], F32, tag="tokfA")
            nc.sync.dma_start(tokfA, tk_d[:, :])
            vslabA = cb.tile([P, NT], F32, tag="vslabA")
            nc.sync.dma_start(vslabA, vs_d[:, :])
            ones_row = cb.tile([1, P], F32, tag="ones_row")
            nc.vector.memset(ones_row, 1.0)
            ones_col = cb.tile([P, 1], F32, tag="ones_col")
            nc.vector.memset(ones_col, 1.0)
            base_sb = cb.tile([1, 8 * (NT + 1)], F32, tag="base")
            nc.vector.memset(base_sb[:, 0:8], 0.0)
            zl = cb.tile([P, NC, 3], F32, tag="zlist")
            nc.vector.memset(zl, 0.0)
            for i in range(NT):
                nc.sync.dma_start(listbufs[i].rearrange("(a p) c -> p a c", p=P), zl)
            if combine == "hostsum":
                zbig = cb.tile([P, H], F32, tag="zbig")
                nc.vector.memset(zbig, 0.0)
                for i in range(2176 // P):
                    nc.sync.dma_start(part_dram[P * i : P * (i + 1), :], zbig)
            rw_t = []
            for h in range(NH):
                t = wp.tile([P, E], F32, tag=f"rw{h}", name=f"rw{h}")
                nc.sync.dma_start(t, rw_d[P * h : P * (h + 1), :])
                rw_t.append(t)

            # batched routing state
            lgA = cb.tile([P, NT, E], F32, tag="lgA")
            valsA = cb.tile([P, NT, 8], F32, tag="valsA")
            idxA = cb.tile([P, NT, 8], U32, tag="idxA")
            PslabA = cb.tile([P, NT, E], F32, tag="PslabA")
            PfullA = cb.tile([P, NT, E], F32, tag="PfullA")
            M_A = cb.tile([P, NT, E], F32, tag="M_A")
            eq1A = cb.tile([P, NT, E], F32, tag="eq1A")
            eq2A = cb.tile([P, NT, E], F32, tag="eq2A")

            # ---- per-tile: transpose x, logits, top2 ----
            for i in range(NT):
                x_sb = wk.tile([P, H], F32, tag="x_sb", bufs=4)
                nc.sync.dma_start(x_sb, x_d[P * i : P * (i + 1), :])
                xT = wk.tile([P, NH, P], F32, tag="xT", bufs=3)
                for h in range(NH):
                    tp = ps.tile([P, P], F32, tag="sps", bufs=4, space="PSUM")
                    nc.tensor.matmul(
                        tp, lhsT=x_sb[:, P * h : P * (h + 1)], rhs=ident, start=True, stop=True
                    )
                    nc.vector.tensor_copy(xT[:, h, :], tp)
                lg_ps = ps.tile([P, E], F32, tag="sps", bufs=4, space="PSUM")
                for h in range(NH):
                    nc.tensor.matmul(
                        lg_ps, lhsT=xT[:, h, :], rhs=rw_t[h], start=(h == 0), stop=(h == NH - 1)
                    )
                nc.vector.tensor_copy(lgA[:, i, :], lg_ps)
                nc.vector.max(out=valsA[:, i, :], in_=lgA[:, i, :])
                nc.vector.max_index(
                    out=idxA[:, i, :], in_max=valsA[:, i, :], in_values=lgA[:, i, :]
                )

            # ---- batched top-2 weights + masks over all NT tiles ----
            idxfA = cb.tile([P, NT, 8], F32, tag="idxfA")
            nc.vector.tensor_copy(idxfA, idxA)
            dA = wk.tile([P, NT], F32, tag="dA")
            nc.vector.tensor_tensor(
                out=dA, in0=valsA[:, :, 1], in1=valsA[:, :, 0], op=OP.subtract
            )
            eA = wk.tile([P, NT], F32, tag="eA")
            nc.scalar.activation(out=eA, in_=dA, func=AF.Exp)
            smA = wk.tile([P, NT], F32, tag="smA")
            nc.vector.tensor_scalar_add(smA, eA, 1.0)
            w1nA = wk.tile([P, NT], F32, tag="w1nA")
            nc.vector.reciprocal(w1nA, smA)
            w2nA = wk.tile([P, NT], F32, tag="w2nA")
            nc.vector.tensor_tensor(out=w2nA, in0=eA, in1=w1nA, op=OP.mult)
            nc.vector.tensor_tensor(
                out=eq1A,
                in0=ecolA,
                in1=idxfA[:, :, 0:1].to_broadcast([P, NT, E]),
                op=OP.is_equal,
            )
            nc.vector.tensor_tensor(
                out=eq2A,
                in0=ecolA,
                in1=idxfA[:, :, 1:2].to_broadcast([P, NT, E]),
                op=OP.is_equal,
            )
            nc.vector.tensor_tensor(out=M_A, in0=eq1A, in1=eq2A, op=OP.add)

            # ---- counts (one matmul) + base prefix chain ----
            cntA_ps = ps.tile([1, NT * E], F32, tag="sps", bufs=4, space="PSUM")
            nc.tensor.matmul(
                cntA_ps,
                lhsT=ones_col,
                rhs=M_A.rearrange("p i e -> p (i e)"),
                start=True,
                stop=True,
            )
            cntA = cb.tile([1, NT * E], F32, tag="cntA")
            nc.vector.tensor_copy(cntA, cntA_ps)
            for i in range(NT):
                nc.vector.tensor_tensor(
                    out=base_sb[:, 8 * (i + 1) : 8 * (i + 2)],
                    in0=base_sb[:, 8 * i : 8 * (i + 1)],
                    in1=cntA[:, 8 * i : 8 * (i + 1)],
                    op=OP.add,
                )

            # ---- per-tile prefix matmuls (tiny, N=8) ----
            for i in range(NT):
                Drow = wk.tile([1, 8], F32, tag="Drow", bufs=4)
                nc.vector.tensor_tensor(
                    out=Drow,
                    in0=base_sb[:, 8 * i : 8 * i + 8],
                    in1=base_sb[:, 8 * (i & ~1) : 8 * (i & ~1) + 8],
                    op=OP.subtract,
                )
                pslab_ps = ps.tile([P, E], F32, tag="sps", bufs=4, space="PSUM")
                nc.tensor.matmul(pslab_ps, lhsT=U, rhs=M_A[:, i, :], start=True, stop=False)
                nc.tensor.matmul(pslab_ps, lhsT=ones_row, rhs=Drow, start=False, stop=True)
                nc.vector.tensor_copy(PslabA[:, i, :], pslab_ps)
                pfull_ps = ps.tile([P, E], F32, tag="sps", bufs=4, space="PSUM")
                nc.tensor.matmul(pfull_ps, lhsT=U, rhs=M_A[:, i, :], start=True, stop=False)
                nc.tensor.matmul(
                    pfull_ps,
                    lhsT=ones_row,
                    rhs=base_sb[:, 8 * i : 8 * i + 8],
                    start=False,
                    stop=True,
                )
                nc.vector.tensor_copy(PfullA[:, i, :], pfull_ps)

            # ---- batched slot/weight/offset algebra ----
            GA = wk.tile([P, NT, E], F32, tag="GA")  # CAP*e + pos_slab
            gec = wk.tile([P, NT, E], F32, tag="gec")
            nc.vector.tensor_scalar(gec, ecolA, float(CAP), None, op0=OP.mult)
            nc.vector.tensor_tensor(out=GA, in0=PslabA, in1=gec, op=OP.add)
            t1 = wk.tile([P, NT, E], F32, tag="t1")
            nc.vector.tensor_tensor(out=t1, in0=GA, in1=eq1A, op=OP.mult)
            off1A = wk.tile([P, NT], F32, tag="off1A")
            nc.vector.reduce_sum(off1A, t1, axis=AX.X)
            t2 = wk.tile([P, NT, E], F32, tag="t2")
            nc.vector.tensor_tensor(out=t2, in0=GA, in1=eq2A, op=OP.mult)
            off2A = wk.tile([P, NT], F32, tag="off2A")
            nc.vector.reduce_sum(off2A, t2, axis=AX.X)
            offpA = wk.tile([P, NT, 2], F32, tag="offpA")
            nc.vector.tensor_copy(offpA[:, :, 0], off1A)
            nc.vector.tensor_copy(offpA[:, :, 1], off2A)
            nc.sync.dma_start(offs_dram.rearrange("(i p) c -> p i c", p=P), offpA)

            selM = wk.tile([P, NT, E], F32, tag="selM")
            nc.vector.tensor_tensor(out=selM, in0=M_A, in1=onehotA, op=OP.mult)
            m_cA = wk.tile([P, NT], F32, tag="m_cA")
            nc.vector.reduce_sum(m_cA, selM, axis=AX.X)
            selP = wk.tile([P, NT, E], F32, tag="selP")
            nc.vector.tensor_tensor(out=selP, in0=PfullA, in1=onehotA, op=OP.mult)
            slot_cA = wk.tile([P, NT], F32, tag="slot_cA")
            nc.vector.reduce_sum(slot_cA, selP, axis=AX.X)
            selS = wk.tile([P, NT, E], F32, tag="selS")
            nc.vector.tensor_tensor(out=selS, in0=PslabA, in1=onehotA, op=OP.mult)
            pos_cA = wk.tile([P, NT], F32, tag="pos_cA")
            nc.vector.reduce_sum(pos_cA, selS, axis=AX.X)
            v_cA = wk.tile([P, NT], F32, tag="v_cA")
            if combine == "hostsum":
                nc.vector.tensor_scalar(
                    v_cA, tokfA, -1.0, 2175.0, op0=OP.mult, op1=OP.add
                )
            else:
                nc.vector.tensor_tensor(out=v_cA, in0=vslabA, in1=pos_cA, op=OP.subtract)
            Wa = wk.tile([P, NT, E], F32, tag="Wa")
            nc.vector.tensor_tensor(
                out=Wa,
                in0=eq1A,
                in1=w1nA.unsqueeze(2).to_broadcast([P, NT, E]),
                op=OP.mult,
            )
            Wb = wk.tile([P, NT, E], F32, tag="Wb")
            nc.vector.tensor_tensor(
                out=Wb,
                in0=eq2A,
                in1=w2nA.unsqueeze(2).to_broadcast([P, NT, E]),
                op=OP.mult,
            )
            Ws = wk.tile([P, NT, E], F32, tag="Ws")
            nc.vector.tensor_tensor(out=Ws, in0=Wa, in1=Wb, op=OP.add)
            selW = wk.tile([P, NT, E], F32, tag="selW")
            nc.vector.tensor_tensor(out=selW, in0=Ws, in1=onehotA, op=OP.mult)
            w_cA = wk.tile([P, NT], F32, tag="w_cA")
            nc.vector.reduce_sum(w_cA, selW, axis=AX.X)
            nmA = wk.tile([P, NT], F32, tag="nmA")
            nc.vector.tensor_scalar(nmA, m_cA, -BIG, BIG, op0=OP.mult, op1=OP.add)
            slot_mA = wk.tile([P, NT], F32, tag="slot_mA")
            nc.vector.tensor_tensor(out=slot_mA, in0=slot_cA, in1=nmA, op=OP.add)
            slot_iA = wk.tile([P, NT], I32, tag="slot_iA")
            nc.vector.tensor_copy(slot_iA, slot_mA)
            payloadA = wk.tile([P, NT, 3], F32, tag="payloadA")
            nc.vector.tensor_copy(payloadA[:, :, 0], tokfA)
            nc.vector.tensor_copy(payloadA[:, :, 1], w_cA)
            nc.vector.tensor_copy(payloadA[:, :, 2], v_cA)
            for i in range(NT):
                nc.gpsimd.indirect_dma_start(
                    out=listbufs[i][:, :],
                    out_offset=IndirectOffsetOnAxis(ap=slot_iA[:, i : i + 1], axis=0),
                    in_=payloadA[:, i, :],
                    in_offset=None,
                    bounds_check=C - 1,
                    oob_is_err=False,
                )

            # ---- weights resident (emitted late; DMAs overlap routing) ----
            w1_t = []
            for h in range(NH):
                t = wp.tile([P, F], BF16, tag=f"w1_{h}", name=f"w1_{h}")
                nc.sync.dma_start(t, w1_d[P * h : P * (h + 1), :])
                w1_t.append(t)
            w2_t = []
            for f in range(NF):
                t = wp.tile([P, H], BF16, tag=f"w2_{f}", name=f"w2_{f}")
                nc.sync.dma_start(t, w2_d[P * f : P * (f + 1), :])
                w2_t.append(t)

            # ---- merge the 16 scatter buffers ----
            lacc = cb.tile([P, NC, 3], F32, tag="lacc")
            for i in range(NT):
                lst = wk.tile([P, NC, 3], F32, tag="lst", bufs=3)
                nc.sync.dma_start(lst, listbufs[i].rearrange("(a p) c -> p a c", p=P))
                if i == 0:
                    nc.vector.tensor_copy(lacc, lst)
                else:
                    nc.vector.tensor_tensor(out=lacc, in0=lacc, in1=lst, op=OP.add)

            # ---- compact MLP ----
            for j in range(NC):
                idx_j = wk.tile([P, 1], I32, tag="idx_j")
                nc.vector.tensor_copy(idx_j, lacc[:, j, 0:1])
                scat_f = wk.tile([P, 1], F32, tag="scat_f")
                sc_hi = 2175.0 if combine == "hostsum" else float(SEND_FULL - 1)
                nc.vector.tensor_scalar(
                    scat_f, lacc[:, j, 2:3], -1.0, sc_hi, op0=OP.mult, op1=OP.add
                )
                scat_i = wk.tile([P, 1], I32, tag="scat_i")
                nc.vector.tensor_copy(scat_i, scat_f)
                xs = wk.tile([P, H], F32, tag="xs")
                nc.gpsimd.indirect_dma_start(
                    out=xs[:, :],
                    out_offset=None,
                    in_=x_d[:, :],
                    in_offset=IndirectOffsetOnAxis(ap=idx_j[:, 0:1], axis=0),
                    bounds_check=T - 1,
                    oob_is_err=False,
                )
                xs_bf = wk.tile([P, H], BF16, tag="xs_bf")
                nc.vector.tensor_copy(xs_bf, xs)
                xsT = wk.tile([P, NH, P], BF16, tag="xsT")
                for h in range(NH):
                    tp = ps.tile([P, P], F32, tag="sps", bufs=4, space="PSUM")
                    nc.tensor.matmul(
                        tp,
                        lhsT=xs_bf[:, P * h : P * (h + 1)],
                        rhs=ident_bf,
                        start=True,
                        stop=True,
                    )
                    nc.vector.tensor_copy(xsT[:, h, :], tp)
                y_ps = ps.tile([P, 1024], F32, tag="yps", bufs=2, space="PSUM")
                for f in range(NF):
                    hT_ps = ps.tile([P, P], F32, tag="sps", bufs=4, space="PSUM")
                    for h in range(NH):
                        nc.tensor.matmul(
                            hT_ps,
                            lhsT=w1_t[h][:, P * f : P * (f + 1)],
                            rhs=xsT[:, h, :],
                            start=(h == 0),
                            stop=(h == NH - 1),
                        )
                    hT = wk.tile([P, P], BF16, tag="hT", bufs=3)
                    nc.scalar.activation(out=hT, in_=hT_ps, func=AF.Silu)
                    nc.tensor.matmul(
                        y_ps[:, 0:512],
                        lhsT=hT,
                        rhs=w2_t[f][:, 0:512],
                        start=(f == 0),
                        stop=(f == NF - 1),
                    )
                    nc.tensor.matmul(
                        y_ps[:, 512:768],
                        lhsT=hT,
                        rhs=w2_t[f][:, 512:768],
                        start=(f == 0),
                        stop=(f == NF - 1),
                    )
                if combine == "hostsum":
                    y_sb = wk.tile([P, H], F32, tag="y_sb")
                    nc.vector.tensor_scalar(
                        y_sb, y_ps[:, 0:H], lacc[:, j, 1:2], None, op0=OP.mult
                    )
                    nc.gpsimd.indirect_dma_start(
                        out=part_dram[:, :],
                        out_offset=IndirectOffsetOnAxis(ap=scat_i[:, 0:1], axis=0),
                        in_=y_sb[:, :],
                        in_offset=None,
                        bounds_check=2175,
                        oob_is_err=False,
                    )
                else:
                    y_sb = wk.tile([P, H], BF16, tag="y_sb")
                    nc.vector.tensor_scalar(
                        y_sb, y_ps[:, 0:H], lacc[:, j, 1:2], None, op0=OP.mult
                    )
                    nc.gpsimd.indirect_dma_start(
                        out=send_dram[:, :],
                        out_offset=IndirectOffsetOnAxis(ap=scat_i[:, 0:1], axis=0),
                        in_=y_sb[:, :],
                        in_offset=None,
                        bounds_check=SEND_FULL - 1,
                        oob_is_err=False,
                    )

            if combine == "hostsum":
                for i in range(T // P):
                    ob = wk.tile([P, H], F32, tag="ob", bufs=3)
                    nc.sync.dma_start(ob, part_dram[P * i : P * (i + 1), :])
                    nc.sync.dma_start(out_d[P * i : P * (i + 1), :], ob)

    if combine != "hostsum":
        # ---- raw tail: AllToAll + owner-side combine (gpsimd only) ----
        NR = SLAB // P
        with (
            nc.semaphore("fin_sem") as fsem,
            nc.sbuf_tensor("r_g1", [P, NR, H], BF16) as g1,
            nc.sbuf_tensor("r_g2", [P, NR, H], BF16) as g2,
            nc.sbuf_tensor("r_osum", [P, H], F32) as osum,
            nc.sbuf_tensor("r_off", [P, NR, 2], F32) as offt,
            nc.sbuf_tensor("r_offi", [P, NR, 2], I32) as offi,
            nc.sbuf_tensor("r_rowi", [P, 1], I32) as rowi,
            nc.sbuf_tensor("r_rowf", [P, 1], F32) as rowf,
            nc.sbuf_tensor("r_sbase", [P, 1], F32) as sbase,
            nc.Block() as blk,
        ):

            @blk.gpsimd
            def _(g: bass.BassEngine):
                sv = [0]

                def fire(inst, n=16):
                    inst.then_inc(fsem, n)
                    sv[0] += n

                def wait():
                    g.wait_ge(fsem, sv[0])

                g.collective_compute(
                    "AllToAll",
                    OP.bypass,
                    replica_groups=[list(range(NCORE))],
                    ins=[send_dram[0:SEND_ROWS, :].opt()],
                    outs=[recv_dram[:, :].opt()],
                ).then_inc(fsem, 1)
                sv[0] += 1
                fire(g.dma_start(sbase[:, :], sb_d[:, :]))
                wait()
                for r in range(NR):
                    g.iota(rowi[:, :], pattern=[[1, 1]], base=P * r, channel_multiplier=1)
                    g.tensor_copy(rowf[:, :], rowi[:, :])
                    g.tensor_tensor(
                        out=rowf[:, :], in0=rowf[:, :], in1=sbase[:, :], op=OP.add
                    )
                    g.tensor_copy(rowi[:, :], rowf[:, :])
                    fire(
                        g.indirect_dma_start(
                            out=offt[:, r, :],
                            out_offset=None,
                            in_=offs_dram[:, :],
                            in_offset=IndirectOffsetOnAxis(ap=rowi[:, 0:1], axis=0),
                            bounds_check=T - 1,
                            oob_is_err=False,
                        )
                    )
                wait()
                g.tensor_copy(offi[:, :, :], offt[:, :, :])
                for r in range(NR):
                    fire(
                        g.indirect_dma_start(
                            out=g1[:, r, :],
                            out_offset=None,
                            in_=recv_dram[:, :],
                            in_offset=IndirectOffsetOnAxis(ap=offi[:, r, 0:1], axis=0),
                            bounds_check=SEND_ROWS - 1,
                            oob_is_err=False,
                        )
                    )
                    fire(
                        g.indirect_dma_start(
                            out=g2[:, r, :],
                            out_offset=None,
                            in_=recv_dram[:, :],
                            in_offset=IndirectOffsetOnAxis(ap=offi[:, r, 1:2], axis=0),
                            bounds_check=SEND_ROWS - 1,
                            oob_is_err=False,
                        )
                    )
                wait()
                for r in range(NR):
                    g.tensor_tensor(
                        out=osum[:, :], in0=g1[:, r, :], in1=g2[:, r, :], op=OP.add
                    )
                    fire(g.dma_start(out_d[P * r : P * (r + 1), :], osum[:, :]))
                    wait()

    _split_attached_waits(nc)
    return nc


def make_in_maps(x, router_w, w1, w2):
    import ml_dtypes

    bf16 = ml_dtypes.bfloat16
    x = np.ascontiguousarray(np.asarray(x, np.float32))
    rw = np.ascontiguousarray(np.asarray(router_w, np.float32))
    w1 = np.asarray(w1, np.float32)
    w2 = np.asarray(w2, np.float32)

    identc = np.eye(P, dtype=np.float32)
    ustrict = np.triu(np.ones((P, P), np.float32), 1)
    ecolA = np.tile(
        np.arange(E, dtype=np.float32)[None, None, :], (P, NT, 1)
    ).reshape(P, NT * E)
    tokfA = (np.arange(P)[:, None] + P * np.arange(NT)[None, :]).astype(np.float32)
    vslabA = np.tile(
        (float(SEND_FULL - 1) - CAP * (np.arange(NT) >> 1))[None, :].astype(np.float32),
        (P, 1),
    )
    in_maps = []
    for c in range(NCORE):
        oh = np.zeros((P, NT, E), np.float32)
        oh[:, :, c] = 1.0
        in_maps.append(
            {
                "x": x,
                "rw": rw,
                "w1c": np.ascontiguousarray(w1[c].astype(bf16)),
                "w2c": np.ascontiguousarray(w2[c].astype(bf16)),
                "identc": identc,
                "ustrict": ustrict,
                "ecolA": ecolA,
                "onehotA": oh.reshape(P, NT * E),
                "tokfA": tokfA,
                "vslabA": vslabA,
                "slabbase": np.full((P, 1), float(SLAB * c), np.float32),
            }
        )
    return in_maps


def gather_output(results, combine=COMBINE):
    if combine == "hostsum":
        out = np.zeros((T, H), np.float64)
        for c in range(NCORE):
            out += results[c]["out"].astype(np.float64)
        return out.astype(np.float32)
    return np.concatenate([results[c]["out"] for c in range(NCORE)], axis=0)


def kernel(x, router_w, w1, w2):
    from concourse.bass_utils import run_bass_kernel_spmd

    combine = COMBINE
    nc = build_nc(combine)
    in_maps = make_in_maps(x, router_w, w1, w2)
    res = run_bass_kernel_spmd(nc, in_maps, list(range(NCORE)))
    return gather_output(res.results, combine)



# revision 2
# speedup vs baseline: 1.6213x; 1.6213x over previous
"""MoE (top-2 of 8 experts) forward on 8 Trainium2 NeuronCores.

Strategy (expert parallel, collective-free):
  - core c owns expert c (w1[c], w2[c] are the only sharded inputs, bf16).
  - every core computes the full routing on device from a host-pretransposed
    xT (fp32 logits via 24 wide matmuls; top-2 via DVE max/max_index;
    counting sort via one strict-triangular prefix matmul + base chain).
  - the core's compact token list (capacity C=640 >= observed max 527) is
    materialized by an indirect-DMA scatter of (token, weight) payloads into
    a DRAM listbuf at the counting-sort slot, then read back.
  - the compact MLP (two grouped GEMMs + silu, bf16 operands / fp32 psum)
    runs batched over all C tokens: GEMM1 streams 640-wide activations per
    (f,h) chunk; GEMM2 accumulates y per 128-token tile over all 24 f chunks.
  - each core writes its compact outputs [C, H] bf16 (routing weight already
    applied) + the (token, weight) list; the host scatter-adds rows into the
    full [2048, 768] output.  No collectives -> no inter-core barrier, no
    start-skew coupling, no all-to-all tail.

kernel(**inputs) -> full [2048, 768] float32 output.
"""
import sys

sys.path.insert(0, "/opt/trn_rl_repo")

import numpy as np

import concourse.bass as bass
import concourse.mybir as mybir
import concourse.tile as tile
from concourse.bass import IndirectOffsetOnAxis

F32 = mybir.dt.float32
BF16 = mybir.dt.bfloat16
I32 = mybir.dt.int32
U32 = mybir.dt.uint32
AF = mybir.ActivationFunctionType
OP = mybir.AluOpType
AX = mybir.AxisListType

T, H, E, K, F = 2048, 768, 8, 2, 3072
P = 128
NCORE = 8
NT = T // P          # 16 token tiles
NH = H // P          # 6 hidden chunks
NF = F // P          # 24 ffn chunks
C = 640              # compact-list capacity per expert (mean 512, obs max 527)
NC = C // P          # 5 compact tiles
BIG = 8192.0

# ---------------------------------------------------------------------------
# This container's walrus cannot attach sem-wait commands to most
# instruction types. Two workarounds (see _split_attached_waits and the
# patched kernel-tail below): waits are moved onto standalone
# EventSemaphore instructions, and the Tile tail drain's waits are
# split across a chain of SP nops.
_MAX_WAITS = 4


def _patched_drain_and_barrier(self, tick_clock, wait_clock):
    from concourse.tile import ScopedClock, VectorClock
    from concourse.tile_sem_assignment import N_PROCS

    g = tick_clock.global_clock
    ticks = [g[p] for p in range(N_PROCS)]
    procs = [p for p in range(N_PROCS) if ticks[p] > 0]
    observed = [0] * N_PROCS
    for i in range(0, len(procs), _MAX_WAITS):
        chunk = set(procs[i : i + _MAX_WAITS])
        part = VectorClock([ticks[p] if p in chunk else 0 for p in range(N_PROCS)])
        nop = self.nc.sync.nop()
        wait_clock.add_sem_waits(
            nop.ins,
            ScopedClock({None: part}),
            ScopedClock({None: VectorClock(list(observed))}),
        )
        for p in chunk:
            observed[p] = ticks[p]
    drain_inst = self.nc.sync.drain()
    wait_clock.add_sem_waits(
        drain_inst.ins,
        ScopedClock({None: g}),
        ScopedClock({None: VectorClock(list(observed))}),
    )
    self.nc.all_engine_barrier()
    assert self.sems is not None
    popped = self.nc._tile_sem_poison_stack.pop()
    assert popped is self._sem_poison
    self.nc.clear_and_free_semaphores(list(self.sems.allocated().values()))
    self.nc.all_engine_barrier()


tile.TileContext._drain_and_barrier = _patched_drain_and_barrier


def _split_attached_waits(nc):
    n = 0
    for f in nc.m.functions:
        for bb in f.blocks:
            new = []
            for inst in bb.instructions:
                si = getattr(inst, "sync_info", None)
                waits = list(si.on_wait) if (si and si.on_wait) else []
                if waits and not isinstance(inst, mybir.InstEventSemaphore):
                    for k, w in enumerate(waits):
                        n += 1
                        new.append(
                            mybir.InstEventSemaphore(
                                name=f"{inst.name}-w{k}",
                                engine=inst.engine,
                                ins=[],
                                outs=[],
                                sync_info=mybir.SyncInfo(on_wait=[w], on_update=[]),
                            )
                        )
                    si.on_wait = []
                new.append(inst)
            bb.instructions[:] = new
    return n


def build_nc():
    nc = bass.Bass(num_devices=NCORE)
    xt_d = nc.declare_dram_parameter("xT", [H, T], F32, isOutput=False)
    xb_d = nc.declare_dram_parameter("xb", [T, H], BF16, isOutput=False)
    rw_d = nc.declare_dram_parameter("rw", [H, E], F32, isOutput=False)
    w1_d = nc.declare_dram_parameter("w1c", [H, F], BF16, isOutput=False)
    w2_d = nc.declare_dram_parameter("w2c", [F, H], BF16, isOutput=False)
    # host-precomputed constants (avoid gpsimd iota/affine at kernel start)
    id_d = nc.declare_dram_parameter("identb", [P, P], BF16, isOutput=False)
    i8_d = nc.declare_dram_parameter("ident8", [8, 8], F32, isOutput=False)
    u_d = nc.declare_dram_parameter("ustrict", [P, P], F32, isOutput=False)
    ec_d = nc.declare_dram_parameter("ecolA", [P, NT * E], F32, isOutput=False)
    oh_d = nc.declare_dram_parameter("onehotA", [P, NT * E], F32, isOutput=False)
    tk_d = nc.declare_dram_parameter("tokfA", [P, NT], F32, isOutput=False)
    outy_d = nc.declare_dram_parameter("out_y", [C, H], BF16, isOutput=True)
    outm_d = nc.declare_dram_parameter("out_m", [C, 2], F32, isOutput=True)

    tc = tile.TileContext(nc)
    with tc:
        with (
            tc.tile_pool(name="dram", bufs=1, space="DRAM") as dr,
            tc.tile_pool(name="consts", bufs=1) as cb,
            tc.tile_pool(name="weights", bufs=1) as wp,
            tc.tile_pool(name="work", bufs=2) as wk,
        ):
            listbuf = dr.tile([C, 2], F32, tag="listbuf", name="listbuf")

            # ---- constants ----
            ident_bf = cb.tile([P, P], BF16, tag="ident_bf")
            nc.sync.dma_start(ident_bf, id_d[:, :])
            i8 = cb.tile([8, 8], F32, tag="i8")
            nc.sync.dma_start(i8, i8_d[:, :])
            U = cb.tile([P, P], F32, tag="ustrict")
            nc.sync.dma_start(U, u_d[:, :])
            ecolA = cb.tile([P, NT, E], F32, tag="ecolA")
            nc.sync.dma_start(ecolA, ec_d[:, :].rearrange("p (i e) -> p i e", e=E))
            onehotA = cb.tile([P, NT, E], F32, tag="onehotA")
            nc.sync.dma_start(onehotA, oh_d[:, :].rearrange("p (i e) -> p i e", e=E))
            tokfA = cb.tile([P, NT], F32, tag="tokfA")
            nc.sync.dma_start(tokfA, tk_d[:, :])
            rw_t = []
            for h in range(NH):
                t = cb.tile([P, E], F32, tag=f"rw{h}", name=f"rw{h}")
                nc.sync.dma_start(t, rw_d[P * h : P * (h + 1), :])
                rw_t.append(t)
            ones_row = cb.tile([1, P], F32, tag="ones_row")
            nc.vector.memset(ones_row, 1.0)
            ones_col = cb.tile([P, 1], F32, tag="ones_col")
            nc.vector.memset(ones_col, 1.0)
            base_sb = cb.tile([1, 8 * (NT + 1)], F32, tag="base")
            nc.vector.memset(base_sb[:, 0:8], 0.0)
            zl = cb.tile([P, NC, 2], F32, tag="zlist")
            nc.vector.memset(zl, 0.0)
            nc.sync.dma_start(listbuf.rearrange("(a p) c -> p a c", p=P), zl)

            # ---- logitsT = rw.T @ xT  (fp32, batched wide) ----
            lgT_sb = cb.tile([8, T], F32, tag="lgT_sb")
            with (
                tc.tile_pool(name="xtp", bufs=1) as xp,
                tc.tile_pool(name="psr", bufs=1, space="PSUM") as pr,
            ):
                lgT_ps = pr.tile([8, T], F32, tag="lgT", space="PSUM")
                xt_t = []
                for h in range(NH):
                    t = xp.tile([P, T], F32, tag=f"xt{h}", name=f"xt{h}")
                    nc.sync.dma_start(t, xt_d[P * h : P * (h + 1), :])
                    xt_t.append(t)
                # emit weight DMAs right after xT so they queue behind it
                w1_t = []
                for h in range(NH):
                    t = wp.tile([P, F], BF16, tag=f"w1_{h}", name=f"w1_{h}")
                    nc.sync.dma_start(t, w1_d[P * h : P * (h + 1), :])
                    w1_t.append(t)
                for h in range(NH):
                    for q in range(4):
                        nc.tensor.matmul(
                            lgT_ps[:, 512 * q : 512 * (q + 1)],
                            lhsT=rw_t[h],
                            rhs=xt_t[h][:, 512 * q : 512 * (q + 1)],
                            start=(h == 0),
                            stop=(h == NH - 1),
                        )
                nc.vector.tensor_copy(lgT_sb, lgT_ps)

            w2_t = []
            for f in range(NF):
                t = wp.tile([P, H], BF16, tag=f"w2_{f}", name=f"w2_{f}")
                nc.sync.dma_start(t, w2_d[P * f : P * (f + 1), :])
                w2_t.append(t)

            with tc.tile_pool(name="ps2", bufs=1, space="PSUM") as p2:
                # ---- transpose logitsT -> [tokens, E]; top-2 ----
                lgA = cb.tile([P, NT, E], F32, tag="lgA")
                valsA = cb.tile([P, NT, 8], F32, tag="valsA")
                idxA = cb.tile([P, NT, 8], U32, tag="idxA")
                for i in range(NT):
                    tp = p2.tile([P, 8], F32, tag="tps", bufs=4, space="PSUM")
                    nc.tensor.matmul(
                        tp,
                        lhsT=lgT_sb[:, P * i : P * (i + 1)],
                        rhs=i8,
                        start=True,
                        stop=True,
                    )
                    nc.vector.tensor_copy(lgA[:, i, :], tp)
                    nc.vector.max(out=valsA[:, i, :], in_=lgA[:, i, :])
                    nc.vector.max_index(
                        out=idxA[:, i, :], in_max=valsA[:, i, :], in_values=lgA[:, i, :]
                    )

                # ---- batched top-2 weights + masks ----
                idxfA = cb.tile([P, NT, 8], F32, tag="idxfA")
                nc.vector.tensor_copy(idxfA, idxA)
                dA = wk.tile([P, NT], F32, tag="dA")
                nc.vector.tensor_tensor(
                    out=dA, in0=valsA[:, :, 1], in1=valsA[:, :, 0], op=OP.subtract
                )
                eA = wk.tile([P, NT], F32, tag="eA")
                nc.scalar.activation(out=eA, in_=dA, func=AF.Exp)
                smA = wk.tile([P, NT], F32, tag="smA")
                nc.vector.tensor_scalar_add(smA, eA, 1.0)
                w1nA = wk.tile([P, NT], F32, tag="w1nA")
                nc.vector.reciprocal(w1nA, smA)
                w2nA = wk.tile([P, NT], F32, tag="w2nA")
                nc.vector.tensor_tensor(out=w2nA, in0=eA, in1=w1nA, op=OP.mult)
                eq1A = cb.tile([P, NT, E], F32, tag="eq1A")
                eq2A = cb.tile([P, NT, E], F32, tag="eq2A")
                M_A = cb.tile([P, NT, E], F32, tag="M_A")
                nc.vector.tensor_tensor(
                    out=eq1A,
                    in0=ecolA,
                    in1=idxfA[:, :, 0:1].to_broadcast([P, NT, E]),
                    op=OP.is_equal,
                )
                nc.vector.tensor_tensor(
                    out=eq2A,
                    in0=ecolA,
                    in1=idxfA[:, :, 1:2].to_broadcast([P, NT, E]),
                    op=OP.is_equal,
                )
                nc.vector.tensor_tensor(out=M_A, in0=eq1A, in1=eq2A, op=OP.add)

                # ---- counts (one matmul) + base prefix chain ----
                cntA_ps = p2.tile([1, NT * E], F32, tag="cnt", space="PSUM")
                nc.tensor.matmul(
                    cntA_ps,
                    lhsT=ones_col,
                    rhs=M_A.rearrange("p i e -> p (i e)"),
                    start=True,
                    stop=True,
                )
                cntA = cb.tile([1, NT * E], F32, tag="cntA")
                nc.vector.tensor_copy(cntA, cntA_ps)
                for i in range(NT):
                    nc.vector.tensor_tensor(
                        out=base_sb[:, 8 * (i + 1) : 8 * (i + 2)],
                        in0=base_sb[:, 8 * i : 8 * (i + 1)],
                        in1=cntA[:, 8 * i : 8 * (i + 1)],
                        op=OP.add,
                    )

                # ---- global slot: strict in-tile prefix + tile base ----
                pf_ps = p2.tile([P, NT * E], F32, tag="pfull", space="PSUM")
                nc.tensor.matmul(
                    pf_ps,
                    lhsT=U,
                    rhs=M_A.rearrange("p i e -> p (i e)"),
                    start=True,
                    stop=False,
                )
                nc.tensor.matmul(
                    pf_ps,
                    lhsT=ones_row,
                    rhs=base_sb[:, 0 : 8 * NT],
                    start=False,
                    stop=True,
                )
                PfullA = cb.tile([P, NT, E], F32, tag="PfullA")
                nc.vector.tensor_copy(PfullA.rearrange("p i e -> p (i e)"), pf_ps)

            # ---- select my expert: in-top2 mask, slot, weight ----
            selM = wk.tile([P, NT, E], F32, tag="selM")
            nc.vector.tensor_tensor(out=selM, in0=M_A, in1=onehotA, op=OP.mult)
            m_cA = wk.tile([P, NT], F32, tag="m_cA")
            nc.vector.reduce_sum(m_cA, selM, axis=AX.X)
            selP = wk.tile([P, NT, E], F32, tag="selP")
            nc.vector.tensor_tensor(out=selP, in0=PfullA, in1=onehotA, op=OP.mult)
            slot_cA = wk.tile([P, NT], F32, tag="slot_cA")
            nc.vector.reduce_sum(slot_cA, selP, axis=AX.X)
            Wa = wk.tile([P, NT, E], F32, tag="Wa")
            nc.vector.tensor_tensor(
                out=Wa,
                in0=eq1A,
                in1=w1nA.unsqueeze(2).to_broadcast([P, NT, E]),
                op=OP.mult,
            )
            Wb = wk.tile([P, NT, E], F32, tag="Wb")
            nc.vector.tensor_tensor(
                out=Wb,
                in0=eq2A,
                in1=w2nA.unsqueeze(2).to_broadcast([P, NT, E]),
                op=OP.mult,
            )
            Ws = wk.tile([P, NT, E], F32, tag="Ws")
            nc.vector.tensor_tensor(out=Ws, in0=Wa, in1=Wb, op=OP.add)
            selW = wk.tile([P, NT, E], F32, tag="selW")
            nc.vector.tensor_tensor(out=selW, in0=Ws, in1=onehotA, op=OP.mult)
            w_cA = wk.tile([P, NT], F32, tag="w_cA")
            nc.vector.reduce_sum(w_cA, selW, axis=AX.X)
            nmA = wk.tile([P, NT], F32, tag="nmA")
            nc.vector.tensor_scalar(nmA, m_cA, -BIG, BIG, op0=OP.mult, op1=OP.add)
            slot_mA = wk.tile([P, NT], F32, tag="slot_mA")
            nc.vector.tensor_tensor(out=slot_mA, in0=slot_cA, in1=nmA, op=OP.add)
            slot_iA = wk.tile([P, NT], I32, tag="slot_iA")
            nc.vector.tensor_copy(slot_iA, slot_mA)
            payloadA = wk.tile([P, NT, 2], F32, tag="payloadA")
            nc.vector.tensor_copy(payloadA[:, :, 0], tokfA)
            nc.vector.tensor_copy(payloadA[:, :, 1], w_cA)
            for i in range(NT):
                nc.gpsimd.indirect_dma_start(
                    out=listbuf[:, :],
                    out_offset=IndirectOffsetOnAxis(ap=slot_iA[:, i : i + 1], axis=0),
                    in_=payloadA[:, i, :],
                    in_offset=None,
                    bounds_check=C - 1,
                    oob_is_err=False,
                )

            # ---- read back compact list; gather + transpose tokens ----
            lacc = cb.tile([P, NC, 2], F32, tag="lacc")
            nc.sync.dma_start(lacc, listbuf.rearrange("(a p) c -> p a c", p=P))
            xsT = cb.tile([P, NH, C], BF16, tag="xsT")
            with tc.tile_pool(name="ps3", bufs=1, space="PSUM") as p3:
                for j in range(NC):
                    idx_j = wk.tile([P, 1], I32, tag="idx_j")
                    nc.vector.tensor_copy(idx_j, lacc[:, j, 0:1])
                    xs = wk.tile([P, H], BF16, tag="xs", bufs=3)
                    nc.gpsimd.indirect_dma_start(
                        out=xs[:, :],
                        out_offset=None,
                        in_=xb_d[:, :],
                        in_offset=IndirectOffsetOnAxis(ap=idx_j[:, 0:1], axis=0),
                        bounds_check=T - 1,
                        oob_is_err=False,
                    )
                    for h in range(NH):
                        tp = p3.tile([P, P], F32, tag="tps", bufs=4, space="PSUM")
                        nc.tensor.matmul(
                            tp,
                            lhsT=xs[:, P * h : P * (h + 1)],
                            rhs=ident_bf,
                            start=True,
                            stop=True,
                        )
                        nc.vector.tensor_copy(xsT[:, h, P * j : P * (j + 1)], tp)

            # ---- GEMM1: h = silu(w1.T @ xsT), batched over all C tokens ----
            h_all = cb.tile([P, NF, C], BF16, tag="h_all")
            with tc.tile_pool(name="ps4", bufs=1, space="PSUM") as p4:
                for f in range(NF):
                    psA = p4.tile([P, 512], F32, tag="psA", bufs=2, space="PSUM")
                    psB = p4.tile([P, C - 512], F32, tag="psB", bufs=2, space="PSUM")
                    for h in range(NH):
                        lw = w1_t[h][:, P * f : P * (f + 1)]
                        nc.tensor.matmul(
                            psA,
                            lhsT=lw,
                            rhs=xsT[:, h, 0:512],
                            start=(h == 0),
                            stop=(h == NH - 1),
                        )
                        nc.tensor.matmul(
                            psB,
                            lhsT=lw,
                            rhs=xsT[:, h, 512:C],
                            start=(h == 0),
                            stop=(h == NH - 1),
                        )
                    nc.scalar.activation(
                        out=h_all[:, f, 0:512], in_=psA, func=AF.Silu
                    )
                    nc.scalar.activation(
                        out=h_all[:, f, 512:C], in_=psB, func=AF.Silu
                    )

                # ---- GEMM2: y = h.T @ w2 per token tile; scale; write out ----
                for j in range(NC):
                    y_ps = p4.tile([P, H], F32, tag="yps", bufs=2, space="PSUM")
                    for f in range(NF):
                        lh = h_all[:, f, P * j : P * (j + 1)]
                        nc.tensor.matmul(
                            y_ps[:, 0:512],
                            lhsT=lh,
                            rhs=w2_t[f][:, 0:512],
                            start=(f == 0),
                            stop=(f == NF - 1),
                        )
                        nc.tensor.matmul(
                            y_ps[:, 512:H],
                            lhsT=lh,
                            rhs=w2_t[f][:, 512:H],
                            start=(f == 0),
                            stop=(f == NF - 1),
                        )
                    y_sb = wk.tile([P, H], BF16, tag="y_sb")
                    nc.vector.tensor_scalar(
                        y_sb, y_ps, lacc[:, j, 1:2], None, op0=OP.mult
                    )
                    nc.sync.dma_start(outy_d[P * j : P * (j + 1), :], y_sb)
            nc.sync.dma_start(outm_d.rearrange("(a p) c -> p a c", p=P), lacc)

    _split_attached_waits(nc)
    return nc


def make_in_maps(x, router_w, w1, w2):
    import ml_dtypes

    bf16 = ml_dtypes.bfloat16
    x = np.ascontiguousarray(np.asarray(x, np.float32))
    rw = np.ascontiguousarray(np.asarray(router_w, np.float32))
    w1 = np.asarray(w1, np.float32)
    w2 = np.asarray(w2, np.float32)

    xT = np.ascontiguousarray(x.T)
    xb = np.ascontiguousarray(x.astype(bf16))
    identb = np.eye(P, dtype=np.float32).astype(bf16)
    ident8 = np.eye(8, dtype=np.float32)
    ustrict = np.triu(np.ones((P, P), np.float32), 1)
    ecolA = np.tile(
        np.arange(E, dtype=np.float32)[None, None, :], (P, NT, 1)
    ).reshape(P, NT * E)
    tokfA = (np.arange(P)[:, None] + P * np.arange(NT)[None, :]).astype(np.float32)
    in_maps = []
    for c in range(NCORE):
        oh = np.zeros((P, NT, E), np.float32)
        oh[:, :, c] = 1.0
        in_maps.append(
            {
                "xT": xT,
                "xb": xb,
                "rw": rw,
                "w1c": np.ascontiguousarray(w1[c].astype(bf16)),
                "w2c": np.ascontiguousarray(w2[c].astype(bf16)),
                "identb": identb,
                "ident8": ident8,
                "ustrict": ustrict,
                "ecolA": ecolA,
                "onehotA": oh.reshape(P, NT * E),
                "tokfA": tokfA,
            }
        )
    return in_maps


def gather_output(results):
    out = np.zeros((T, H), np.float32)
    for c in range(NCORE):
        y = np.asarray(results[c]["out_y"], np.float32)
        m = np.asarray(results[c]["out_m"], np.float32)
        tok = m[:, 0].astype(np.int64)
        np.add.at(out, tok, y)
    return out


def kernel(x, router_w, w1, w2):
    from concourse.bass_utils import run_bass_kernel_spmd

    nc = build_nc()
    in_maps = make_in_maps(x, router_w, w1, w2)
    res = run_bass_kernel_spmd(nc, in_maps, list(range(NCORE)))
    return gather_output(res.results)


# revision 10
# speedup vs baseline: 2.1436x; 1.3221x over previous
"""MoE (top-2 of 8 experts) forward on 8 Trainium2 NeuronCores.

Strategy (expert parallel, collective-free):
  - core c owns expert c (w1[c], w2[c] are the only sharded inputs, bf16).
  - every core computes the full routing on device from a host-pretransposed
    xT (fp32 logits via 24 wide matmuls; top-2 via DVE max/max_index;
    counting sort via one strict-triangular prefix matmul + base chain).
  - the core's compact token list (capacity C=640 >= observed max 527) is
    materialized by an indirect-DMA scatter of (token, weight) payloads into
    a DRAM listbuf at the counting-sort slot, then read back.
  - the compact MLP (two grouped GEMMs + silu, bf16 operands / fp32 psum)
    runs batched over all C tokens: GEMM1 streams 640-wide activations per
    (f,h) chunk; GEMM2 accumulates y per 128-token tile over all 24 f chunks.
  - each core writes its compact outputs [C, H] bf16 (routing weight already
    applied) + the (token, weight) list; the host scatter-adds rows into the
    full [2048, 768] output.  No collectives -> no inter-core barrier, no
    start-skew coupling, no all-to-all tail.

kernel(**inputs) -> full [2048, 768] float32 output.
"""
import sys

sys.path.insert(0, "/opt/trn_rl_repo")

import numpy as np

import concourse.bass as bass
import concourse.mybir as mybir
import concourse.tile as tile
from concourse.bass import IndirectOffsetOnAxis

F32 = mybir.dt.float32
BF16 = mybir.dt.bfloat16
I32 = mybir.dt.int32
U32 = mybir.dt.uint32
AF = mybir.ActivationFunctionType
OP = mybir.AluOpType
AX = mybir.AxisListType

T, H, E, K, F = 2048, 768, 8, 2, 3072
P = 128
NCORE = 8
NT = T // P          # 16 token tiles
NH = H // P          # 6 hidden chunks
NF = F // P          # 24 ffn chunks
C = 640              # compact-list capacity per expert (mean 512, obs max 527)
NC = C // P          # 5 compact tiles
BIG = 8192.0

# ---------------------------------------------------------------------------
# This container's walrus cannot attach sem-wait commands to most
# instruction types. Two workarounds (see _split_attached_waits and the
# patched kernel-tail below): waits are moved onto standalone
# EventSemaphore instructions, and the Tile tail drain's waits are
# split across a chain of SP nops.
_MAX_WAITS = 4


def _patched_drain_and_barrier(self, tick_clock, wait_clock):
    from concourse.tile import ScopedClock, VectorClock
    from concourse.tile_sem_assignment import N_PROCS

    g = tick_clock.global_clock
    ticks = [g[p] for p in range(N_PROCS)]
    procs = [p for p in range(N_PROCS) if ticks[p] > 0]
    observed = [0] * N_PROCS
    for i in range(0, len(procs), _MAX_WAITS):
        chunk = set(procs[i : i + _MAX_WAITS])
        part = VectorClock([ticks[p] if p in chunk else 0 for p in range(N_PROCS)])
        nop = self.nc.sync.nop()
        wait_clock.add_sem_waits(
            nop.ins,
            ScopedClock({None: part}),
            ScopedClock({None: VectorClock(list(observed))}),
        )
        for p in chunk:
            observed[p] = ticks[p]
    drain_inst = self.nc.sync.drain()
    wait_clock.add_sem_waits(
        drain_inst.ins,
        ScopedClock({None: g}),
        ScopedClock({None: VectorClock(list(observed))}),
    )
    self.nc.all_engine_barrier()
    assert self.sems is not None
    popped = self.nc._tile_sem_poison_stack.pop()
    assert popped is self._sem_poison
    self.nc.clear_and_free_semaphores(list(self.sems.allocated().values()))
    self.nc.all_engine_barrier()


tile.TileContext._drain_and_barrier = _patched_drain_and_barrier


def _split_attached_waits(nc):
    n = 0
    for f in nc.m.functions:
        for bb in f.blocks:
            new = []
            for inst in bb.instructions:
                si = getattr(inst, "sync_info", None)
                waits = list(si.on_wait) if (si and si.on_wait) else []
                if waits and not isinstance(inst, mybir.InstEventSemaphore):
                    for k, w in enumerate(waits):
                        n += 1
                        new.append(
                            mybir.InstEventSemaphore(
                                name=f"{inst.name}-w{k}",
                                engine=inst.engine,
                                ins=[],
                                outs=[],
                                sync_info=mybir.SyncInfo(on_wait=[w], on_update=[]),
                            )
                        )
                    si.on_wait = []
                new.append(inst)
            bb.instructions[:] = new
    return n


def build_nc():
    nc = bass.Bass(num_devices=NCORE)
    xt_d = nc.declare_dram_parameter("xT", [H, T], F32, isOutput=False)
    xb_d = nc.declare_dram_parameter("xb", [T, H], BF16, isOutput=False)
    rw_d = nc.declare_dram_parameter("rw", [H, E], F32, isOutput=False)
    w1_d = nc.declare_dram_parameter("w1c", [H, F], BF16, isOutput=False)
    w2_d = nc.declare_dram_parameter("w2c", [F, H], BF16, isOutput=False)
    # host-precomputed constants (avoid gpsimd iota/affine at kernel start)
    id_d = nc.declare_dram_parameter("identb", [P, P], BF16, isOutput=False)
    i8_d = nc.declare_dram_parameter("ident8", [8, 8], F32, isOutput=False)
    u_d = nc.declare_dram_parameter("ustrict", [P, P], F32, isOutput=False)
    ec_d = nc.declare_dram_parameter("ecolA", [P, NT * E], F32, isOutput=False)
    oh_d = nc.declare_dram_parameter("onehotA", [P, NT * E], F32, isOutput=False)
    th_d = nc.declare_dram_parameter("tokhiA", [P, NT], F32, isOutput=False)
    tl_d = nc.declare_dram_parameter("tokloA", [P, NT], F32, isOutput=False)
    io_d = nc.declare_dram_parameter("iota640", [P, C], F32, isOutput=False)
    outy_d = nc.declare_dram_parameter("out_y", [C, H], BF16, isOutput=True)
    outm_d = nc.declare_dram_parameter("out_m", [C, 2], F32, isOutput=True)

    tc = tile.TileContext(nc)
    with tc:
        with (
            tc.tile_pool(name="dram", bufs=1, space="DRAM") as dr,
            tc.tile_pool(name="consts", bufs=1) as cb,
            tc.tile_pool(name="weights", bufs=1) as wp,
            tc.tile_pool(name="work", bufs=2) as wk,
        ):
            # ---- constants ----
            ident_bf = cb.tile([P, P], BF16, tag="ident_bf")
            nc.sync.dma_start(ident_bf, id_d[:, :])
            i8 = cb.tile([8, 8], F32, tag="i8")
            nc.sync.dma_start(i8, i8_d[:, :])
            U = cb.tile([P, P], F32, tag="ustrict")
            nc.sync.dma_start(U, u_d[:, :])
            ecolA = cb.tile([P, NT, E], F32, tag="ecolA")
            nc.sync.dma_start(ecolA, ec_d[:, :].rearrange("p (i e) -> p i e", e=E))
            onehotA = cb.tile([P, NT, E], F32, tag="onehotA")
            nc.sync.dma_start(onehotA, oh_d[:, :].rearrange("p (i e) -> p i e", e=E))
            tokhiA = cb.tile([P, NT], F32, tag="tokhiA")
            nc.sync.dma_start(tokhiA, th_d[:, :])
            tokloA = cb.tile([P, NT], F32, tag="tokloA")
            nc.sync.dma_start(tokloA, tl_d[:, :])
            iota640 = cb.tile([P, C], F32, tag="iota640")
            nc.sync.dma_start(iota640, io_d[:, :])
            rw_t = []
            for h in range(NH):
                t = cb.tile([P, E], F32, tag=f"rw{h}", name=f"rw{h}")
                nc.sync.dma_start(t, rw_d[P * h : P * (h + 1), :])
                rw_t.append(t)
            ones_row = cb.tile([1, P], F32, tag="ones_row")
            nc.vector.memset(ones_row, 1.0)
            ones_col = cb.tile([P, 1], F32, tag="ones_col")
            nc.vector.memset(ones_col, 1.0)
            base_sb = cb.tile([1, 8 * (NT + 1)], F32, tag="base")
            nc.vector.memset(base_sb[:, 0:8], 0.0)

            # ---- logitsT = rw.T @ xT  (fp32, batched wide) ----
            lgT_sb = cb.tile([8, T], F32, tag="lgT_sb")
            with (
                tc.tile_pool(name="xtp", bufs=1) as xp,
                tc.tile_pool(name="psr", bufs=1, space="PSUM") as pr,
            ):
                lgT_ps = pr.tile([8, T], F32, tag="lgT", space="PSUM")
                xt_t = []
                for h in range(NH):
                    t = xp.tile([P, T], F32, tag=f"xt{h}", name=f"xt{h}")
                    nc.sync.dma_start(t, xt_d[P * h : P * (h + 1), :])
                    xt_t.append(t)
                # emit weight DMAs right after xT so they queue behind it
                w1_t = []
                for h in range(NH):
                    t = wp.tile([P, F], BF16, tag=f"w1_{h}", name=f"w1_{h}")
                    nc.sync.dma_start(t, w1_d[P * h : P * (h + 1), :])
                    w1_t.append(t)
                for h in range(NH):
                    for q in range(4):
                        nc.tensor.matmul(
                            lgT_ps[:, 512 * q : 512 * (q + 1)],
                            lhsT=rw_t[h],
                            rhs=xt_t[h][:, 512 * q : 512 * (q + 1)],
                            start=(h == 0),
                            stop=(h == NH - 1),
                        )
                nc.vector.tensor_copy(lgT_sb, lgT_ps)

            w2_t = []
            for f in range(NF):
                t = wp.tile([P, H], BF16, tag=f"w2_{f}", name=f"w2_{f}")
                nc.sync.dma_start(t, w2_d[P * f : P * (f + 1), :])
                w2_t.append(t)

            with tc.tile_pool(name="ps2", bufs=1, space="PSUM") as p2:
                # ---- transpose logitsT -> [tokens, E]; top-2 ----
                lgA = cb.tile([P, NT, E], F32, tag="lgA")
                valsA = cb.tile([P, NT, 8], F32, tag="valsA")
                idxA = cb.tile([P, NT, 8], U32, tag="idxA")
                for i in range(NT):
                    tp = p2.tile([P, 8], F32, tag="tps", bufs=4, space="PSUM")
                    nc.tensor.matmul(
                        tp,
                        lhsT=lgT_sb[:, P * i : P * (i + 1)],
                        rhs=i8,
                        start=True,
                        stop=True,
                    )
                    nc.vector.tensor_copy(lgA[:, i, :], tp)
                    nc.vector.max(out=valsA[:, i, :], in_=lgA[:, i, :])
                    nc.vector.max_index(
                        out=idxA[:, i, :], in_max=valsA[:, i, :], in_values=lgA[:, i, :]
                    )

                # ---- batched top-2 weights + masks ----
                idxfA = cb.tile([P, NT, 8], F32, tag="idxfA")
                nc.vector.tensor_copy(idxfA, idxA)
                dA = wk.tile([P, NT], F32, tag="dA")
                nc.vector.tensor_tensor(
                    out=dA, in0=valsA[:, :, 1], in1=valsA[:, :, 0], op=OP.subtract
                )
                eA = wk.tile([P, NT], F32, tag="eA")
                nc.scalar.activation(out=eA, in_=dA, func=AF.Exp)
                smA = wk.tile([P, NT], F32, tag="smA")
                nc.vector.tensor_scalar_add(smA, eA, 1.0)
                w1nA = wk.tile([P, NT], F32, tag="w1nA")
                nc.vector.reciprocal(w1nA, smA)
                w2nA = wk.tile([P, NT], F32, tag="w2nA")
                nc.vector.tensor_tensor(out=w2nA, in0=eA, in1=w1nA, op=OP.mult)
                eq1A = cb.tile([P, NT, E], F32, tag="eq1A")
                eq2A = cb.tile([P, NT, E], F32, tag="eq2A")
                M_A = cb.tile([P, NT, E], F32, tag="M_A")
                nc.vector.tensor_tensor(
                    out=eq1A,
                    in0=ecolA,
                    in1=idxfA[:, :, 0:1].to_broadcast([P, NT, E]),
                    op=OP.is_equal,
                )
                nc.vector.tensor_tensor(
                    out=eq2A,
                    in0=ecolA,
                    in1=idxfA[:, :, 1:2].to_broadcast([P, NT, E]),
                    op=OP.is_equal,
                )
                nc.vector.tensor_tensor(out=M_A, in0=eq1A, in1=eq2A, op=OP.add)

                # ---- counts (one matmul) + base prefix chain ----
                cntA_ps = p2.tile([1, NT * E], F32, tag="cnt", space="PSUM")
                nc.tensor.matmul(
                    cntA_ps,
                    lhsT=ones_col,
                    rhs=M_A.rearrange("p i e -> p (i e)"),
                    start=True,
                    stop=True,
                )
                cntA = cb.tile([1, NT * E], F32, tag="cntA")
                nc.vector.tensor_copy(cntA, cntA_ps)
                for i in range(NT):
                    nc.vector.tensor_tensor(
                        out=base_sb[:, 8 * (i + 1) : 8 * (i + 2)],
                        in0=base_sb[:, 8 * i : 8 * (i + 1)],
                        in1=cntA[:, 8 * i : 8 * (i + 1)],
                        op=OP.add,
                    )

                # ---- global slot: strict in-tile prefix + tile base ----
                pf_ps = p2.tile([P, NT * E], F32, tag="pfull", space="PSUM")
                nc.tensor.matmul(
                    pf_ps,
                    lhsT=U,
                    rhs=M_A.rearrange("p i e -> p (i e)"),
                    start=True,
                    stop=False,
                )
                nc.tensor.matmul(
                    pf_ps,
                    lhsT=ones_row,
                    rhs=base_sb[:, 0 : 8 * NT],
                    start=False,
                    stop=True,
                )
                PfullA = cb.tile([P, NT, E], F32, tag="PfullA")
                nc.vector.tensor_copy(PfullA.rearrange("p i e -> p (i e)"), pf_ps)

            # ---- select my expert: in-top2 mask, slot, weight ----
            selM = wk.tile([P, NT, E], F32, tag="selM")
            nc.vector.tensor_tensor(out=selM, in0=M_A, in1=onehotA, op=OP.mult)
            m_cA = wk.tile([P, NT], F32, tag="m_cA")
            nc.vector.reduce_sum(m_cA, selM, axis=AX.X)
            selP = wk.tile([P, NT, E], F32, tag="selP")
            nc.vector.tensor_tensor(out=selP, in0=PfullA, in1=onehotA, op=OP.mult)
            slot_cA = wk.tile([P, NT], F32, tag="slot_cA")
            nc.vector.reduce_sum(slot_cA, selP, axis=AX.X)
            Wa = wk.tile([P, NT, E], F32, tag="Wa")
            nc.vector.tensor_tensor(
                out=Wa,
                in0=eq1A,
                in1=w1nA.unsqueeze(2).to_broadcast([P, NT, E]),
                op=OP.mult,
            )
            Wb = wk.tile([P, NT, E], F32, tag="Wb")
            nc.vector.tensor_tensor(
                out=Wb,
                in0=eq2A,
                in1=w2nA.unsqueeze(2).to_broadcast([P, NT, E]),
                op=OP.mult,
            )
            Ws = wk.tile([P, NT, E], F32, tag="Ws")
            nc.vector.tensor_tensor(out=Ws, in0=Wa, in1=Wb, op=OP.add)
            selW = wk.tile([P, NT, E], F32, tag="selW")
            nc.vector.tensor_tensor(out=selW, in0=Ws, in1=onehotA, op=OP.mult)
            w_cA = wk.tile([P, NT], F32, tag="w_cA")
            nc.vector.reduce_sum(w_cA, selW, axis=AX.X)
            nmA = wk.tile([P, NT], F32, tag="nmA")
            nc.vector.tensor_scalar(nmA, m_cA, -BIG, BIG, op0=OP.mult, op1=OP.add)
            slot_mA = wk.tile([P, NT], F32, tag="slot_mA")
            nc.vector.tensor_tensor(out=slot_mA, in0=slot_cA, in1=nmA, op=OP.add)
            payloadA = wk.tile([P, NT, 3], BF16, tag="payloadA")
            nc.vector.tensor_copy(payloadA[:, :, 0], tokhiA)
            nc.vector.tensor_copy(payloadA[:, :, 1], tokloA)
            nc.vector.tensor_copy(payloadA[:, :, 2], w_cA)

            # ---- compact list via one-hot selection matmuls (no DRAM trip):
            # list[c, s] = sum_t payload[t, c] * [slot(t) == s]
            laccT = cb.tile([P, NC, 3], F32, tag="laccT")
            tok_fA = cb.tile([P, NC], F32, tag="tok_fA")
            with tc.tile_pool(name="psL", bufs=1, space="PSUM") as pL:
                list_ps = pL.tile([3, C], F32, tag="list", space="PSUM")
                for i in range(NT):
                    sel = wk.tile([P, C], BF16, tag="sel", bufs=3)
                    nc.vector.tensor_tensor(
                        out=sel,
                        in0=slot_mA[:, i : i + 1].to_broadcast([P, C]),
                        in1=iota640,
                        op=OP.is_equal,
                    )
                    nc.tensor.matmul(
                        list_ps[:, 0:512],
                        lhsT=payloadA[:, i, :],
                        rhs=sel[:, 0:512],
                        start=(i == 0),
                        stop=(i == NT - 1),
                    )
                    nc.tensor.matmul(
                        list_ps[:, 512:C],
                        lhsT=payloadA[:, i, :],
                        rhs=sel[:, 512:C],
                        start=(i == 0),
                        stop=(i == NT - 1),
                    )
                list_sb = cb.tile([3, C], F32, tag="list_sb")
                nc.vector.tensor_copy(list_sb, list_ps)
                for j in range(NC):
                    tpl = pL.tile([P, 3], F32, tag="tpl", bufs=2, space="PSUM")
                    nc.tensor.matmul(
                        tpl,
                        lhsT=list_sb[:, P * j : P * (j + 1)],
                        rhs=i8[0:3, 0:3],
                        start=True,
                        stop=True,
                    )
                    nc.vector.tensor_copy(laccT[:, j, :], tpl)
                nc.vector.tensor_scalar(
                    tok_fA, laccT[:, :, 0], 256.0, None, op0=OP.mult
                )
                nc.vector.tensor_tensor(
                    out=tok_fA, in0=tok_fA, in1=laccT[:, :, 1], op=OP.add
                )

            xsT = cb.tile([P, NH, C], BF16, tag="xsT")
            with tc.tile_pool(name="ps3", bufs=1, space="PSUM") as p3:
                for j in range(NC):
                    idx_j = wk.tile([P, 1], I32, tag="idx_j")
                    nc.vector.tensor_copy(idx_j, tok_fA[:, j : j + 1])
                    xs = wk.tile([P, H], BF16, tag="xs", bufs=3)
                    nc.gpsimd.indirect_dma_start(
                        out=xs[:, :],
                        out_offset=None,
                        in_=xb_d[:, :],
                        in_offset=IndirectOffsetOnAxis(ap=idx_j[:, 0:1], axis=0),
                        bounds_check=T - 1,
                        oob_is_err=False,
                    )
                    for h in range(NH):
                        tp = p3.tile([P, P], F32, tag="tps", bufs=4, space="PSUM")
                        nc.tensor.matmul(
                            tp,
                            lhsT=xs[:, P * h : P * (h + 1)],
                            rhs=ident_bf,
                            start=True,
                            stop=True,
                        )
                        nc.vector.tensor_copy(xsT[:, h, P * j : P * (j + 1)], tp)

            # ---- GEMM1: h = silu(w1.T @ xsT), batched over all C tokens ----
            h_all = cb.tile([P, NF, C], BF16, tag="h_all")
            with tc.tile_pool(name="ps4", bufs=1, space="PSUM") as p4:
                for f in range(NF):
                    psA = p4.tile([P, 512], F32, tag="psA", bufs=2, space="PSUM")
                    psB = p4.tile([P, C - 512], F32, tag="psB", bufs=2, space="PSUM")
                    for h in range(NH):
                        lw = w1_t[h][:, P * f : P * (f + 1)]
                        nc.tensor.matmul(
                            psA,
                            lhsT=lw,
                            rhs=xsT[:, h, 0:512],
                            start=(h == 0),
                            stop=(h == NH - 1),
                        )
                        nc.tensor.matmul(
                            psB,
                            lhsT=lw,
                            rhs=xsT[:, h, 512:C],
                            start=(h == 0),
                            stop=(h == NH - 1),
                        )
                    nc.scalar.activation(
                        out=h_all[:, f, 0:512], in_=psA, func=AF.Silu
                    )
                    nc.scalar.activation(
                        out=h_all[:, f, 512:C], in_=psB, func=AF.Silu
                    )

                # ---- GEMM2: y = h.T @ w2 per token tile; scale; write out ----
                for j in range(NC):
                    y_ps = p4.tile([P, H], F32, tag="yps", bufs=2, space="PSUM")
                    for f in range(NF):
                        lh = h_all[:, f, P * j : P * (j + 1)]
                        nc.tensor.matmul(
                            y_ps[:, 0:512],
                            lhsT=lh,
                            rhs=w2_t[f][:, 0:512],
                            start=(f == 0),
                            stop=(f == NF - 1),
                        )
                        nc.tensor.matmul(
                            y_ps[:, 512:H],
                            lhsT=lh,
                            rhs=w2_t[f][:, 512:H],
                            start=(f == 0),
                            stop=(f == NF - 1),
                        )
                    y_sb = wk.tile([P, H], BF16, tag="y_sb")
                    nc.vector.tensor_scalar(
                        y_sb, y_ps, laccT[:, j, 2:3], None, op0=OP.mult
                    )
                    nc.sync.dma_start(outy_d[P * j : P * (j + 1), :], y_sb)
            outm_sb = cb.tile([P, NC, 2], F32, tag="outm_sb")
            nc.vector.tensor_copy(outm_sb[:, :, 0], tok_fA)
            nc.vector.tensor_copy(outm_sb[:, :, 1], laccT[:, :, 2])
            nc.sync.dma_start(outm_d.rearrange("(a p) c -> p a c", p=P), outm_sb)

    _split_attached_waits(nc)
    return nc


def make_in_maps(x, router_w, w1, w2):
    import ml_dtypes

    bf16 = ml_dtypes.bfloat16
    x = np.ascontiguousarray(np.asarray(x, np.float32))
    rw = np.ascontiguousarray(np.asarray(router_w, np.float32))
    w1 = np.asarray(w1, np.float32)
    w2 = np.asarray(w2, np.float32)

    xT = np.ascontiguousarray(x.T)
    xb = np.ascontiguousarray(x.astype(bf16))
    identb = np.eye(P, dtype=np.float32).astype(bf16)
    ident8 = np.eye(8, dtype=np.float32)
    ustrict = np.triu(np.ones((P, P), np.float32), 1)
    ecolA = np.tile(
        np.arange(E, dtype=np.float32)[None, None, :], (P, NT, 1)
    ).reshape(P, NT * E)
    tokA = np.arange(P)[:, None] + P * np.arange(NT)[None, :]
    tokhiA = (tokA // 256).astype(np.float32)
    tokloA = (tokA % 256).astype(np.float32)
    iota640 = np.tile(np.arange(C, dtype=np.float32)[None, :], (P, 1))
    in_maps = []
    for c in range(NCORE):
        oh = np.zeros((P, NT, E), np.float32)
        oh[:, :, c] = 1.0
        in_maps.append(
            {
                "xT": xT,
                "xb": xb,
                "rw": rw,
                "w1c": np.ascontiguousarray(w1[c].astype(bf16)),
                "w2c": np.ascontiguousarray(w2[c].astype(bf16)),
                "identb": identb,
                "ident8": ident8,
                "ustrict": ustrict,
                "ecolA": ecolA,
                "onehotA": oh.reshape(P, NT * E),
                "tokhiA": tokhiA,
                "tokloA": tokloA,
                "iota640": iota640,
            }
        )
    return in_maps


def gather_output(results):
    out = np.zeros((T, H), np.float32)
    for c in range(NCORE):
        y = np.asarray(results[c]["out_y"], np.float32)
        m = np.asarray(results[c]["out_m"], np.float32)
        tok = m[:, 0].astype(np.int64)
        np.add.at(out, tok, y)
    return out


def kernel(x, router_w, w1, w2):
    from concourse.bass_utils import run_bass_kernel_spmd

    nc = build_nc()
    in_maps = make_in_maps(x, router_w, w1, w2)
    res = run_bass_kernel_spmd(nc, in_maps, list(range(NCORE)))
    return gather_output(res.results)
